# revision 44
# baseline (speedup 1.0000x reference)
"""BinaryConv2d (3x3, stride 1, pad 1) on 8 TRN2 NeuronCores.

Data-parallel: batch 32 sharded 4-per-core; weight/bias replicated.

v3: 5 taps run as bf16 matmuls (448 cols) and 4 taps run as 2 fp8
DoubleRow matmuls, each packing a vertical tap pair (dh=-1,dh=0 at the
same dw) into the PE's 2-weights-per-cell mode (~1.44x bf16 rate at
this free-dim).  The fp8 side reads a 64-wide e4m3 slab so the pair's
slot stride (one row, 64 B) meets the DoubleRow step%16==0 constraint;
the slot dim is spliced into the AP by hand ([64,2] over the same rows
as the row dim).  Host-side: x is pre-padded/cast twice (bf16 [58,58]
and fp8e4m3 [58,64]); weights are pre-binarized +/-1 (exact in both
dtypes).  fp8 quantization of 4/9 taps gives rel err ~1.8e-2 (vs 2e-2
budget), measured exactly in numpy against the same reference; the
output is stored bf16 and upcast to f32 on the host (rel 1.836e-2,
also verified exactly in numpy), halving store traffic.

All input prep is host-side, so the device does only matmuls, one
bias-add tensor_scalar per PSUM group, and DMA.  Input DMAs ride the SP
hardware-DGE ring; the single fused weight blob and the output DMAs
ride the Activation ring.  Warmup dummies complete the HAM clock ramp
(~3.9us) so real matmuls always start at 2.4 GHz regardless of
DMA-arrival jitter; the last chunk is split into two 4-row PSUM groups,
the final one draining over the by-then-idle SP ring, so the
post-last-matmul tail is a half-size tensor_scalar + transfer.
"""

import numpy as np
import ml_dtypes
from contextlib import ExitStack

import concourse.bass as bass
import concourse.bacc as bacc
import concourse.mybir as mybir
import concourse.tile as tile
from concourse.bass_utils import run_bass_kernel_spmd

N_CORES = 8
N_BATCH = 32
N_PER_CORE = N_BATCH // N_CORES  # 4
C_IN = 128
C_OUT = 256
H = W = 56
HP = H + 2
WP = W + 2
WP8 = 64             # fp8 slab row pitch (DoubleRow step%16==0)
NROWS = 8            # output rows per matmul chunk
NCHUNK = H // NROWS  # 7
NWARM = 38           # dummy matmuls: complete the HAM ramp before work

f32 = mybir.dt.float32
bf16 = mybir.dt.bfloat16
f8 = mybir.dt.float8e4
ALU = mybir.AluOpType
DR = mybir.MatmulPerfMode.DoubleRow

# taps 0..8 = (dh,dw) row-major over dh,dw in {-1,0,1}
# fp8 vertical pairs: (tap0,tap3) at dw=-1 and (tap1,tap4) at dw=0
FP8_PAIRS = [(-1, -1), (-1, 0)]      # (dh of top tap, dw) per pair
BF16_TAPS = [(-1, 1), (0, 1), (1, -1), (1, 0), (1, 1)]


def build_program() -> bass.Bass:
    nc = bacc.Bacc("TRN2", target_bir_lowering=False, debug=False)
    x = nc.dram_tensor("x", [N_PER_CORE, C_IN, HP, WP], bf16,
                       kind="ExternalInput")
    x8 = nc.dram_tensor("x8", [N_PER_CORE, C_IN, HP, WP8], f8,
                        kind="ExternalInput")
    # One fused +/-1 fp8 weight blob (exact in e4m3): first 1024 B/part =
    # DoubleRow pairs [pair, slot, half, o], then 1280 B/part = the 5
    # bf16-moving taps [j, half, o] (mixed fp8-stationary matmul).
    w = nc.dram_tensor("w", [C_IN, 2304], f8, kind="ExternalInput")
    b = nc.dram_tensor("b", [128, 2], f32, kind="ExternalInput")
    y = nc.dram_tensor("y", [N_PER_CORE, C_OUT, H, W], bf16,
                       kind="ExternalOutput")

    with tile.TileContext(nc) as tc, ExitStack() as ctx:
        singles = ctx.enter_context(tc.tile_pool(name="singles", bufs=1))
        xslab = ctx.enter_context(tc.tile_pool(name="xslab", bufs=8))
        x8slab = ctx.enter_context(tc.tile_pool(name="x8slab", bufs=8))
        psum_mm = ctx.enter_context(
            tc.tile_pool(name="psum_mm", bufs=8, space="PSUM")
        )
        outp = ctx.enter_context(tc.tile_pool(name="outp", bufs=6))

        wall = singles.tile([128, 2304], f8)
        wD = wall[:, 0:1024].rearrange(
            "p (pair s h o) -> p pair s h o", pair=2, s=2, h=2, o=128)
        wT = wall[:, 1024:2304].rearrange(
            "p (j h o) -> p j h o", j=5, h=2, o=128)
        bsb = singles.tile([128, 2], f32)
        warm_w = singles.tile([128, 128], bf16)

        slabs = {}

        def slab_dma(n, c):
            xc8 = x8slab.tile([128, 10, WP8], f8, name="xc8", tag="xc8")
            nc.sync.dma_start(
                out=xc8, in_=x8.ap()[n, :, c * NROWS:c * NROWS + 10, :]
            )
            xc = xslab.tile([128, 10, WP], bf16, name="xc", tag="xc")
            nc.sync.dma_start(
                out=xc, in_=x.ap()[n, :, c * NROWS:c * NROWS + 10, :]
            )
            slabs[(n, c)] = (xc, xc8)

        # GPSIMD is otherwise idle and its queue clears the start barrier
        # first, so the PE warmup (and with it the HAM clock ramp) starts
        # ~0.5us earlier than a DVE-produced tile would allow.
        nc.gpsimd.memset(warm_w, 0.0)

        # Startup DMAs: slabs on the SP ring; weights+bias on the ACT ring.
        slab_dma(0, 0)
        nc.scalar.dma_start(out=wall, in_=w.ap())
        nc.scalar.dma_start(out=bsb, in_=b.ap())

        # ---- PE warmup: complete the HAM 4096-cycle activity ramp ----
        wp = psum_mm.tile([128, 128], f32, tag="ps")
        for k in range(NWARM):
            nc.tensor.matmul(wp, lhsT=warm_w, rhs=warm_w,
                             start=(k == 0), stop=(k == NWARM - 1))

        for c in range(1, NCHUNK):
            slab_dma(0, c)
        slab_dma(1, 0)

        def dr_rhs(xc8, dh, dw, nr):
            """[128, 2slot, nr, 56] view of the fp8 slab: slot 1 is the
            next row down (the dh+1 tap of the vertical pair)."""
            s = xc8[:, dh + 1: dh + 1 + nr, dw + 1: dw + 1 + W].copy()
            s.ap = [s.ap[0], [WP8, 2]] + s.ap[1:]
            return s

        # ---- main loop ----
        # The very last chunk is split into two 4-row PSUM groups so the
        # post-final-matmul drain (tensor_scalar + DGE config + transfer)
        # is half-size; its store rides the by-then-empty SP ring.
        work = [(n, c, 0, NROWS) for n in range(N_PER_CORE)
                for c in range(NCHUNK)]
        work[-1:] = [(N_PER_CORE - 1, NCHUNK - 1, 0, 6),
                     (N_PER_CORE - 1, NCHUNK - 1, 6, 2)]

        for wi, (n, c, r0, nr) in enumerate(work):
            h0 = c * NROWS + r0
            xc, xc8 = slabs[(n, c)]
            last = wi == len(work) - 1
            ob = outp.tile([128, 2, nr, W], bf16, name="ob", tag="ob")
            for half in range(2):
                ps = psum_mm.tile([128, nr, W], f32, name="ps", tag="ps")
                for p, (dh, dw) in enumerate(FP8_PAIRS):
                    nc.tensor.matmul(
                        ps,
                        lhsT=wD[:, p, :, half, :],
                        rhs=dr_rhs(xc8, r0 + dh, dw, nr),
                        start=(p == 0),
                        stop=False,
                        perf_mode=DR,
                    )
                for j, (dh, dw) in enumerate(BF16_TAPS):
                    rhs = xc[:, r0 + dh + 1: r0 + dh + 1 + nr,
                             dw + 1: dw + 1 + W]
                    nc.tensor.matmul(
                        ps,
                        lhsT=wT[:, j, half, :],
                        rhs=rhs,
                        start=False,
                        stop=(j == len(BF16_TAPS) - 1),
                    )
                if last and half == 1:
                    obz = singles.tile([128, nr, W], bf16)
                    nc.vector.tensor_scalar(
                        out=obz, in0=ps,
                        scalar1=bsb[:, half:half + 1], scalar2=None,
                        op0=ALU.add,
                    )
                    nc.sync.dma_start(
                        out=y.ap()[n, half * 128:(half + 1) * 128,
                                   h0:h0 + nr, :],
                        in_=obz,
                    )
                    continue
                nc.vector.tensor_scalar(
                    out=ob[:, half], in0=ps,
                    scalar1=bsb[:, half:half + 1], scalar2=None,
                    op0=ALU.add,
                )
                if last:
                    nc.scalar.dma_start(
                        out=y.ap()[n, half * 128:(half + 1) * 128,
                                   h0:h0 + nr, :],
                        in_=ob[:, half],
                    )
            if r0 == 0:
                idx = n * NCHUNK + c
                if idx + 8 < N_PER_CORE * NCHUNK:
                    slab_dma((idx + 8) // NCHUNK, (idx + 8) % NCHUNK)
            if not last:
                nc.scalar.dma_start(
                    out=y.ap()[n].rearrange(
                        "(h o) r w -> o h r w", h=2
                    )[:, :, h0:h0 + nr, :],
                    in_=ob,
                )
    nc.compile()
    return nc


def host_prep(x, weight, bias):
    """Pad+cast x (bf16 and fp8), binarize+transpose weights, bias."""
    x = np.asarray(x, dtype=np.float32)
    xp = np.zeros((N_BATCH, C_IN, HP, WP), dtype=ml_dtypes.bfloat16)
    xp[:, :, 1:1 + H, 1:1 + W] = x.astype(ml_dtypes.bfloat16)
    xp8 = np.zeros((N_BATCH, C_IN, HP, WP8), dtype=ml_dtypes.float8_e4m3)
    xp8[:, :, 1:1 + H, 1:1 + W] = x.astype(ml_dtypes.float8_e4m3)

    w = np.asarray(weight, dtype=np.float32)
    wbin = np.where(np.clip(w, -1.0, 1.0) >= 0, 1.0, -1.0).astype(np.float32)
    # [O, I, 3, 3] -> [half, o, i, tap]
    w4 = wbin.reshape(2, 128, C_IN, 9)
    # bf16-side taps (fp8 +/-1 stationary), [i, j, half, o]
    bt = [(dh + 1) * 3 + (dw + 1) for dh, dw in BF16_TAPS]
    w5 = np.ascontiguousarray(
        w4[:, :, :, bt].transpose(2, 3, 0, 1)).astype(ml_dtypes.float8_e4m3)
    # fp8 pairs, [i, pair, slot, half, o]; slot 1 = tap one row down
    w8 = np.empty((C_IN, 2, 2, 2, 128), dtype=ml_dtypes.float8_e4m3)
    for p, (dh, dw) in enumerate(FP8_PAIRS):
        for s in range(2):
            tap = (dh + s + 1) * 3 + (dw + 1)
            w8[:, p, s] = w4[:, :, :, tap].transpose(2, 0, 1)
    wcat = np.concatenate(
        [w8.reshape(C_IN, 1024), w5.reshape(C_IN, 1280)], axis=1)
    b2 = np.ascontiguousarray(
        np.asarray(bias, dtype=np.float32).reshape(2, 128).T)
    return xp, xp8, np.ascontiguousarray(wcat), b2


def run(x, weight, bias, trace=False):
    """Returns (out [32,256,56,56] f32, BassKernelResults)."""
    nc = build_program()
    xp, xp8, wcat, b2 = host_prep(x, weight, bias)
    in_maps = [
        {
            "x": xp[i * N_PER_CORE:(i + 1) * N_PER_CORE],
            "x8": xp8[i * N_PER_CORE:(i + 1) * N_PER_CORE],
            "w": wcat,
            "b": b2,
        }
        for i in range(N_CORES)
    ]
    res = run_bass_kernel_spmd(
        nc, in_maps, core_ids=list(range(N_CORES)), trace=trace
    )
    out = np.concatenate([r["y"] for r in res.results],
                         axis=0).astype(np.float32)
    return out, res


def kernel(x, weight, bias):
    out, _ = run(x, weight, bias)
    return out
